# revision 12
# baseline (speedup 1.0000x reference)
"""DownSample (depthwise FIR [1,3,3,1]^2/64 pad-2, then 3x3 stride-2 conv + bias)
as a Trainium2 Bass kernel, data-parallel over batch across 8 NeuronCores.

Per core (2 batch images), per (batch, ic-chunk) block:
  ACT : f32 -> bf16 ingest cast into a row-padded buffer
  DVE : vertical FIR [1,3,3,1] as three [1,1]-box cascades (all 2x mode)
        horizontal [1,2,1] as u = V[g-1]+V[g+1] (2x) and M = u + 2V
  ACT : the 2V scaled copy + PSUM evacuation with fused bias
  PE  : y[oc, oh, ow] = sum over (ic-chunk, kh in 3, q in 4) of
        w4[kh,q,ic,oc]^T @ M[ic, 2oh+kh, 2ow+q-1]   (stride-2 strided rhs APs)
        where w4 = conv_w(w, [1,1]) (horizontal [1,1] folded into weights),
        24 accumulating N=512 matmuls per PSUM bank, 8 banks.
Host folds [1,1]_h and the 1/64 FIR norm into w4.
"""

import numpy as np
import ml_dtypes

import concourse.bass as bass
import concourse.mybir as mybir
import concourse.tile as tile
from concourse.bass_utils import run_bass_kernel_spmd
from concourse.vector_clock import ScopedClock, VectorClock

# problem geometry (hardcoded per contract)
B_FULL, C, H, W = 16, 256, 64, 64
OC, OH, OW = 256, 32, 32
N_CORES = 8
BPC = B_FULL // N_CORES      # batches per core
KH, KQ = 3, 4                # folded conv taps (rows x cols)
NCH = C // 128               # input-channel chunks
NOB = OC // 128              # output-channel blocks
WV = W + 4                   # padded row width of V/u/v2/M buffers

F32 = mybir.dt.float32
BF16 = mybir.dt.bfloat16
COPY = mybir.ActivationFunctionType.Copy
IDENT = mybir.ActivationFunctionType.Identity


class SplitDrainTileContext(tile.TileContext):
    """walrus codegen caps sync-wait commands per instruction; the kernel-tail
    drain waits once per proc lane, which overflows once several DMA queues
    are used. Emit one single-wait SP nop per pending proc first; SP executes
    in order, so the drain itself then needs no waits."""

    def _drain_and_barrier(self, tick_clock, wait_clock):
        g = tick_clock.global_clock
        n = len(g)
        for p in range(n):
            if g[p] > 0:
                vec = [g[q] if q == p else 0 for q in range(n)]
                nop = self.nc.sync.nop()
                wait_clock.add_sem_waits(nop.ins, ScopedClock({None: VectorClock(vec)}))
        self.nc.sync.drain()
        self.nc.all_engine_barrier()
        assert self.sems is not None
        popped = self.nc._tile_sem_poison_stack.pop()
        assert popped is self._sem_poison
        self.nc.clear_and_free_semaphores(list(self.sems.allocated().values()))
        self.nc.all_engine_barrier()


def _split_excess_waits(nc: bass.Bass, max_waits: int = 1) -> None:
    """The TRN2 ISA (and walrus codegen) allows at most 2 sync-wait commands
    per instruction (1 for matmul), but Tile's wait assignment can attach more. Spill excess
    waits onto same-engine nops inserted immediately before the instruction —
    the engine executes them first, so the precondition set is identical."""
    for fn in nc.m.functions:
        for bb in fn.blocks:
            out = []
            changed = False
            for ins in bb.instructions:
                si = ins.sync_info
                waits = list(si.on_wait) if si else []
                if len(waits) > max_waits:
                    changed = True
                    excess, keep = waits[:-max_waits], waits[-max_waits:]
                    for k in range(0, len(excess), max_waits):
                        nop = mybir.InstNoOp(
                            name=f"{ins.name}-wsplit{k}", ins=[], outs=[],
                            engine=ins.engine)
                        nop.sync_info = mybir.SyncInfo(
                            on_wait=excess[k:k + max_waits], on_update=[])
                        nc.register_instruction(nop, overwrite=True)
                        out.append(nop)
                    ins.sync_info = mybir.SyncInfo(
                        on_wait=keep, on_update=list(si.on_update))
                out.append(ins)
            if changed:
                bb.instructions = out


def build_program(reps: int = 1) -> bass.Bass:
    """One-core program; run SPMD on 8 cores. `reps` unrolls the body for
    steady-state timing (outputs are rewritten identically each rep)."""
    nc = bass.Bass()
    xin = nc.declare_dram_parameter("x", [BPC, C, H, W], F32, isOutput=False)
    win = nc.declare_dram_parameter("w", [128, KH * KQ * NCH * NOB * 128], BF16,
                                    isOutput=False)
    bin_ = nc.declare_dram_parameter("b", [NOB, 128], F32, isOutput=False)
    yout = nc.declare_dram_parameter("y", [BPC, OC, OH, OW], F32, isOutput=True)

    with SplitDrainTileContext(nc) as tc:
        with tc.tile_pool(name="const", bufs=1) as cpool, \
             tc.tile_pool(name="xf", bufs=4) as xfpool, \
             tc.tile_pool(name="fir", bufs=1) as f1pool, \
             tc.tile_pool(name="fir2", bufs=2) as f2pool, \
             tc.tile_pool(name="mimg", bufs=2 * NCH) as mpool, \
             tc.tile_pool(name="osb", bufs=2) as opool, \
             tc.tile_pool(name="ps", bufs=8, space="PSUM") as pspool:

            w_sb = cpool.tile([128, KH * KQ * NCH * NOB * 128], BF16)
            nc.sync.dma_start(out=w_sb[:], in_=win[:])
            bias_sb = cpool.tile([128, NOB], F32)
            nc.sync.dma_start(out=bias_sb[:], in_=bin_[:].rearrange("o p -> p o"))

            blocks = [(b, c) for b in range(BPC) for c in range(NCH)]
            NB = len(blocks)

            for rep in range(reps):
                # x ingest: DMA + f32->bf16 cast, split in row-halves so the
                # first block's data lands fast (chunks spread over DMA lanes)
                def emit_load(i):
                    b, c = blocks[i]
                    xb = f2pool.tile([128, (H + 4) * W], BF16, tag="xb",
                                     name=f"xb{rep}_{i}")
                    xb3 = xb[:].rearrange("p (h w) -> p h w", w=W)
                    nc.vector.memset(xb3[:, 0:2, :], 0.0)
                    nc.vector.memset(xb3[:, H + 2:H + 4, :], 0.0)
                    for half in range(2):
                        r0 = half * (H // 2)
                        xf = xfpool.tile([128, H * W // 2], F32, tag="xf",
                                         name=f"xf{rep}_{i}_{half}")
                        nc.sync.dma_start(
                            out=xf[:],
                            in_=xin[b, c * 128:(c + 1) * 128, r0:r0 + H // 2, :]
                            .rearrange("c h w -> c (h w)"),
                        )
                        nc.scalar.activation(
                            xb3[:, 2 + r0:2 + r0 + H // 2, :],
                            xf[:].rearrange("p (h w) -> p h w", w=W), COPY)
                    return xb3

                # FIR compute for one block -> M image
                def emit_fir(i, xb3):
                    # vertical FIR [1,3,3,1] as a 3x [1,1]-box cascade
                    c1 = f1pool.tile([128, (H + 3) * W], BF16, tag="c1",
                                     name=f"c1_{rep}_{i}")
                    c13 = c1[:].rearrange("p (h w) -> p h w", w=W)
                    nc.vector.tensor_add(c13, xb3[:, 0:H + 3, :],
                                         xb3[:, 1:H + 4, :])
                    c2 = f1pool.tile([128, (H + 2) * W], BF16, tag="c2",
                                     name=f"c2_{rep}_{i}")
                    c23 = c2[:].rearrange("p (h w) -> p h w", w=W)
                    nc.vector.tensor_add(c23, c13[:, 0:H + 2, :],
                                         c13[:, 1:H + 3, :])
                    # V cols 2..65 = data, cols 0,1,66,67 zero margins
                    vb = f2pool.tile([128, (H + 1) * WV], BF16, tag="v",
                                     name=f"v_{rep}_{i}")
                    v3 = vb[:].rearrange("p (h w) -> p h w", w=WV)
                    nc.vector.memset(v3[:, :, 0:2], 0.0)
                    nc.vector.memset(v3[:, :, W + 2:W + 4], 0.0)
                    nc.vector.tensor_add(v3[:, 0:H + 1, 2:2 + W],
                                         c23[:, 0:H + 1, :], c23[:, 1:H + 2, :])
                    # horizontal [1,2,1]: M[g] = V[g-1] + 2V[g] + V[g+1],
                    # g = -1..64 stored at col g+1 (0..65)
                    v2b = f2pool.tile([128, (H + 1) * WV], BF16, tag="v2",
                                      name=f"v2_{rep}_{i}")
                    v23 = v2b[:].rearrange("p (h w) -> p h w", w=WV)
                    nc.scalar.activation(v23[:, :, 0:W + 2],
                                         v3[:, :, 1:W + 3], COPY, scale=2.0)
                    ub = f1pool.tile([128, (H + 1) * WV], BF16, tag="u",
                                     name=f"u_{rep}_{i}")
                    u3 = ub[:].rearrange("p (h w) -> p h w", w=WV)
                    nc.vector.tensor_add(u3[:, :, 0:W + 2], v3[:, :, 0:W + 2],
                                         v3[:, :, 2:W + 4])
                    mb_ = mpool.tile([128, (H + 1) * WV], BF16, tag="m",
                                     name=f"m_{rep}_{i}")
                    m3 = mb_[:].rearrange("p (h w) -> p h w", w=WV)
                    nc.vector.tensor_add(m3[:, :, 0:W + 2], u3[:, :, 0:W + 2],
                                         v23[:, :, 0:W + 2])
                    return m3

                # pre-allocate all 8 PSUM banks for this rep
                pss = {(b, ocb): [pspool.tile([128, 512], F32, tag="ps",
                                              name=f"ps{rep}_{b}_{ocb}_{g}")
                                  for g in range(2)]
                       for b in range(BPC) for ocb in range(NOB)}

                xb3s = {}
                xb3s[0] = emit_load(0)
                if NB > 1:
                    xb3s[1] = emit_load(1)

                def emit_warmup(xb3, n_mm):
                    # PE warmup: garbage matmuls into a bank the real taps
                    # later clear (start=True). Gated on this block's xb so
                    # they spread through the FIR lead-in, keeping HAM warm.
                    wout = pss[(0, 0)][0][:].rearrange("p (h w) -> p h w", w=OW)
                    rhs = xb3[:, 0:16, 0:32]
                    for _ in range(n_mm):
                        nc.tensor.matmul(wout, w_sb[:, 0:128], rhs,
                                         start=True, stop=True)

                mimg = {}
                for i in range(NB):
                    if rep == 0:
                        emit_warmup(xb3s[i], 12)
                    mimg[blocks[i]] = emit_fir(i, xb3s[i])
                    if i + 2 < NB:
                        xb3s[i + 2] = emit_load(i + 2)

                # ---------- conv stage ----------
                # ic-chunk outermost so PE starts after the first chunk's FIR
                for b in range(BPC):
                    for ci in range(NCH):
                        for ocb in range(NOB):
                            for kh in range(KH):
                                for q in range(KQ):
                                    widx = ((kh * KQ + q) * NCH + ci) * NOB + ocb
                                    lhsT = w_sb[:, widx * 128:(widx + 1) * 128]
                                    first = ci == 0 and kh == 0 and q == 0
                                    last = (ci == NCH - 1 and kh == KH - 1
                                            and q == KQ - 1)
                                    for grp in range(2):
                                        r0 = kh + 32 * grp
                                        rhs = mimg[(b, ci)][:, r0:r0 + 31:2,
                                                            q:q + 63:2]
                                        out3 = pss[(b, ocb)][grp][:].rearrange(
                                            "p (h w) -> p h w", w=OW)
                                        nc.tensor.matmul(out3, lhsT, rhs,
                                                         start=first, stop=last)
                    for ocb in range(NOB):
                        osb = opool.tile([128, OH * OW], F32, tag="osb",
                                         name=f"osb{rep}_{b}_{ocb}")
                        for grp in range(2):
                            nc.scalar.activation(
                                osb[:, grp * 512:(grp + 1) * 512],
                                pss[(b, ocb)][grp][:],
                                IDENT, bias=bias_sb[:, ocb:ocb + 1])
                        nc.sync.dma_start(
                            out=yout[b, ocb * 128:(ocb + 1) * 128, :, :]
                            .rearrange("c h w -> c (h w)"),
                            in_=osb[:])
    _split_excess_waits(nc)
    return nc


def prep_weights(w: np.ndarray) -> np.ndarray:
    """w [256,256,3,3] f32 -> [128, 48*128] bf16 lhsT tiles.
    Folds horizontal [1,1] and the 1/64 FIR normalization:
    w4[q] coefficients multiply M[2ow+q-1]."""
    w = np.asarray(w, np.float32)
    w4 = np.zeros((OC, C, KH, KQ), np.float32)
    w4[:, :, :, 0] = w[:, :, :, 0]
    w4[:, :, :, 1] = w[:, :, :, 0] + w[:, :, :, 1]
    w4[:, :, :, 2] = w[:, :, :, 1] + w[:, :, :, 2]
    w4[:, :, :, 3] = w[:, :, :, 2]
    w4 *= 1.0 / 64.0
    # -> [kh, q, c_chunk, ocb, ic(128), oc(128)]
    t = w4.reshape(NOB, 128, NCH, 128, KH, KQ).transpose(4, 5, 2, 0, 3, 1)
    t = np.ascontiguousarray(t).reshape(KH * KQ * NCH * NOB, 128, 128)
    return t.transpose(1, 0, 2).reshape(128, -1).astype(ml_dtypes.bfloat16)


_NC_CACHE: dict[int, bass.Bass] = {}


def _get_nc(reps: int = 1) -> bass.Bass:
    if reps not in _NC_CACHE:
        _NC_CACHE[reps] = build_program(reps)
    return _NC_CACHE[reps]


def make_in_maps(x: np.ndarray, w: np.ndarray, b: np.ndarray):
    wp = prep_weights(w)
    bp = np.asarray(b, np.float32).reshape(NOB, 128)
    return [
        {"x": np.ascontiguousarray(np.asarray(x, np.float32)[i * BPC:(i + 1) * BPC]),
         "w": wp, "b": bp}
        for i in range(N_CORES)
    ]


def kernel(x: np.ndarray, w: np.ndarray, b: np.ndarray) -> np.ndarray:
    nc = _get_nc(1)
    res = run_bass_kernel_spmd(nc, make_in_maps(x, w, b), list(range(N_CORES)))
    return np.concatenate([res.results[i]["y"] for i in range(N_CORES)],
                          axis=0).astype(np.float32)


# revision 14
# speedup vs baseline: 2.8027x; 2.8027x over previous
"""DownSample (depthwise FIR [1,3,3,1]^2/64 pad-2, then 3x3 stride-2 conv + bias)
as a Trainium2 Bass kernel, data-parallel over batch across 8 NeuronCores.

Per core (2 batch images), per (batch, ic-chunk) block:
  ACT : f32 -> bf16 ingest cast into a row-padded buffer
  DVE : vertical FIR [1,3,3,1] as three [1,1]-box cascades (all 2x mode)
        horizontal [1,2,1] as u = V[g-1]+V[g+1] (2x) and M = u + 2V
  ACT : the 2V scaled copy + PSUM evacuation with fused bias
  PE  : y[oc, oh, ow] = sum over (ic-chunk, kh in 3, q in 4) of
        w4[kh,q,ic,oc]^T @ M[ic, 2oh+kh, 2ow+q-1]   (stride-2 strided rhs APs)
        where w4 = conv_w(w, [1,1]) (horizontal [1,1] folded into weights),
        24 accumulating N=512 matmuls per PSUM bank, 8 banks.
Host folds [1,1]_h and the 1/64 FIR norm into w4.
"""

import numpy as np
import ml_dtypes

import concourse.bass as bass
import concourse.mybir as mybir
import concourse.tile as tile
from concourse.bass_utils import run_bass_kernel_spmd
from concourse.vector_clock import ScopedClock, VectorClock

# problem geometry (hardcoded per contract)
B_FULL, C, H, W = 16, 256, 64, 64
OC, OH, OW = 256, 32, 32
N_CORES = 8
BPC = B_FULL // N_CORES      # batches per core
KH, KQ = 3, 4                # folded conv taps (rows x cols)
NCH = C // 128               # input-channel chunks
NOB = OC // 128              # output-channel blocks
WV = W + 4                   # padded row width of V/u/v2/M buffers

F32 = mybir.dt.float32
BF16 = mybir.dt.bfloat16
COPY = mybir.ActivationFunctionType.Copy
IDENT = mybir.ActivationFunctionType.Identity


class SplitDrainTileContext(tile.TileContext):
    """walrus codegen caps sync-wait commands per instruction; the kernel-tail
    drain waits once per proc lane, which overflows once several DMA queues
    are used. Emit one single-wait SP nop per pending proc first; SP executes
    in order, so the drain itself then needs no waits."""

    def _drain_and_barrier(self, tick_clock, wait_clock):
        g = tick_clock.global_clock
        n = len(g)
        for p in range(n):
            if g[p] > 0:
                vec = [g[q] if q == p else 0 for q in range(n)]
                nop = self.nc.sync.nop()
                wait_clock.add_sem_waits(nop.ins, ScopedClock({None: VectorClock(vec)}))
        self.nc.sync.drain()
        self.nc.all_engine_barrier()
        assert self.sems is not None
        popped = self.nc._tile_sem_poison_stack.pop()
        assert popped is self._sem_poison
        self.nc.clear_and_free_semaphores(list(self.sems.allocated().values()))
        self.nc.all_engine_barrier()


def _split_excess_waits(nc: bass.Bass, max_waits: int = 1) -> None:
    """The TRN2 ISA (and walrus codegen) allows at most 2 sync-wait commands
    per instruction (1 for matmul), but Tile's wait assignment can attach more. Spill excess
    waits onto same-engine nops inserted immediately before the instruction —
    the engine executes them first, so the precondition set is identical."""
    for fn in nc.m.functions:
        for bb in fn.blocks:
            out = []
            changed = False
            for ins in bb.instructions:
                si = ins.sync_info
                waits = list(si.on_wait) if si else []
                if len(waits) > max_waits:
                    changed = True
                    excess, keep = waits[:-max_waits], waits[-max_waits:]
                    for k in range(0, len(excess), max_waits):
                        nop = mybir.InstNoOp(
                            name=f"{ins.name}-wsplit{k}", ins=[], outs=[],
                            engine=ins.engine)
                        nop.sync_info = mybir.SyncInfo(
                            on_wait=excess[k:k + max_waits], on_update=[])
                        nc.register_instruction(nop, overwrite=True)
                        out.append(nop)
                    ins.sync_info = mybir.SyncInfo(
                        on_wait=keep, on_update=list(si.on_update))
                out.append(ins)
            if changed:
                bb.instructions = out


def build_program(reps: int = 1) -> bass.Bass:
    """One-core program; run SPMD on 8 cores. `reps` unrolls the body for
    steady-state timing (outputs are rewritten identically each rep)."""
    nc = bass.Bass()
    xin = nc.declare_dram_parameter("x", [BPC, C, H, W], F32, isOutput=False)
    win = nc.declare_dram_parameter("w", [128, KH * KQ * NCH * NOB * 128], BF16,
                                    isOutput=False)
    bin_ = nc.declare_dram_parameter("b", [NOB, 128], F32, isOutput=False)
    yout = nc.declare_dram_parameter("y", [BPC, OC, OH, OW], F32, isOutput=True)

    with SplitDrainTileContext(nc) as tc:
        with tc.tile_pool(name="const", bufs=1) as cpool, \
             tc.tile_pool(name="xf", bufs=4) as xfpool, \
             tc.tile_pool(name="fir", bufs=1) as f1pool, \
             tc.tile_pool(name="fir2", bufs=2) as f2pool, \
             tc.tile_pool(name="mimg", bufs=2 * NCH) as mpool, \
             tc.tile_pool(name="osb", bufs=2) as opool, \
             tc.tile_pool(name="ps", bufs=8, space="PSUM") as pspool:

            w_sb = cpool.tile([128, KH * KQ * NCH * NOB * 128], BF16)
            nc.sync.dma_start(out=w_sb[:], in_=win[:])
            bias_sb = cpool.tile([128, NOB], F32)
            nc.sync.dma_start(out=bias_sb[:], in_=bin_[:].rearrange("o p -> p o"))

            blocks = [(b, c) for b in range(BPC) for c in range(NCH)]
            NB = len(blocks)

            for rep in range(reps):
                # x ingest: DMA + f32->bf16 cast, split in row-halves so the
                # first half's M image is ready fast (chunks spread over lanes)
                def emit_load(i):
                    b, c = blocks[i]
                    xb = f2pool.tile([128, (H + 4) * W], BF16, tag="xb",
                                     name=f"xb{rep}_{i}")
                    xb3 = xb[:].rearrange("p (h w) -> p h w", w=W)
                    nc.vector.memset(xb3[:, 0:2, :], 0.0)
                    nc.vector.memset(xb3[:, H + 2:H + 4, :], 0.0)
                    # chunk a: x rows 0..33 -> xb rows 2..35
                    # chunk b: x rows 34..63 -> xb rows 36..65
                    for (r0, nr) in ((0, 34), (34, 30)):
                        xf = xfpool.tile([128, nr * W], F32, tag="xf",
                                         name=f"xf{rep}_{i}_{r0}")
                        nc.sync.dma_start(
                            out=xf[:],
                            in_=xin[b, c * 128:(c + 1) * 128, r0:r0 + nr, :]
                            .rearrange("c h w -> c (h w)"),
                        )
                        nc.scalar.activation(
                            xb3[:, 2 + r0:2 + r0 + nr, :],
                            xf[:].rearrange("p (h w) -> p h w", w=W), COPY)
                    return xb3

                # FIR for one block, row-half `half` -> M tile for psum grp
                # `half`. Half A covers M rows 0..32, half B rows 32..64
                # (row 32 shared: grp0 reads Ma, grp1 reads Mb).
                # Buffers c1/c2/v/u/v2 are shared across halves (disjoint row
                # ranges); B reads a few boundary rows written by A.
                def emit_fir_half(i, half, st):
                    if half == 0:
                        c1 = f1pool.tile([128, (H + 3) * W], BF16, tag="c1",
                                         name=f"c1_{rep}_{i}")
                        c13 = c1[:].rearrange("p (h w) -> p h w", w=W)
                        # c1 rows 0..34
                        nc.vector.tensor_add(c13[:, 0:35, :], st["xb3"][:, 0:35, :],
                                             st["xb3"][:, 1:36, :])
                        c2 = f1pool.tile([128, (H + 2) * W], BF16, tag="c2",
                                         name=f"c2_{rep}_{i}")
                        c23 = c2[:].rearrange("p (h w) -> p h w", w=W)
                        # c2 rows 0..33
                        nc.vector.tensor_add(c23[:, 0:34, :], c13[:, 0:34, :],
                                             c13[:, 1:35, :])
                        vb = f2pool.tile([128, (H + 1) * WV], BF16, tag="v",
                                         name=f"v_{rep}_{i}")
                        v3 = vb[:].rearrange("p (h w) -> p h w", w=WV)
                        nc.vector.memset(v3[:, :, 0:2], 0.0)
                        nc.vector.memset(v3[:, :, W + 2:W + 4], 0.0)
                        # V rows 0..32 (cols 2..65 = data)
                        nc.vector.tensor_add(v3[:, 0:33, 2:2 + W],
                                             c23[:, 0:33, :], c23[:, 1:34, :])
                        v2b = f2pool.tile([128, (H + 1) * WV], BF16, tag="v2",
                                          name=f"v2_{rep}_{i}")
                        v23 = v2b[:].rearrange("p (h w) -> p h w", w=WV)
                        nc.scalar.activation(v23[:, 0:33, 0:W + 2],
                                             v3[:, 0:33, 1:W + 3], COPY, scale=2.0)
                        ub = f1pool.tile([128, (H + 1) * WV], BF16, tag="u",
                                         name=f"u_{rep}_{i}")
                        u3 = ub[:].rearrange("p (h w) -> p h w", w=WV)
                        nc.vector.tensor_add(u3[:, 0:33, 0:W + 2],
                                             v3[:, 0:33, 0:W + 2],
                                             v3[:, 0:33, 2:W + 4])
                        ma = mpool.tile([128, 33 * WV], BF16, tag="m",
                                        name=f"ma_{rep}_{i}")
                        ma3 = ma[:].rearrange("p (h w) -> p h w", w=WV)
                        nc.vector.tensor_add(ma3[:, :, 0:W + 2],
                                             u3[:, 0:33, 0:W + 2],
                                             v23[:, 0:33, 0:W + 2])
                        st.update(c13=c13, c23=c23, v3=v3, v23=v23, u3=u3)
                        return ma3
                    else:
                        c13, c23, v3, v23, u3 = (st["c13"], st["c23"], st["v3"],
                                                 st["v23"], st["u3"])
                        # c1 rows 35..66
                        nc.vector.tensor_add(c13[:, 35:67, :],
                                             st["xb3"][:, 35:67, :],
                                             st["xb3"][:, 36:68, :])
                        # c2 rows 34..65 (c1 row 34 from half A)
                        nc.vector.tensor_add(c23[:, 34:66, :], c13[:, 34:66, :],
                                             c13[:, 35:67, :])
                        # V rows 33..64 (c2 row 33 from half A)
                        nc.vector.tensor_add(v3[:, 33:65, 2:2 + W],
                                             c23[:, 33:65, :], c23[:, 34:66, :])
                        nc.scalar.activation(v23[:, 33:65, 0:W + 2],
                                             v3[:, 33:65, 1:W + 3], COPY,
                                             scale=2.0)
                        nc.vector.tensor_add(u3[:, 33:65, 0:W + 2],
                                             v3[:, 33:65, 0:W + 2],
                                             v3[:, 33:65, 2:W + 4])
                        mb = mpool.tile([128, 33 * WV], BF16, tag="m",
                                        name=f"mb_{rep}_{i}")
                        mb3 = mb[:].rearrange("p (h w) -> p h w", w=WV)
                        # M rows 32..64 (u/v2 row 32 from half A)
                        nc.vector.tensor_add(mb3[:, :, 0:W + 2],
                                             u3[:, 32:65, 0:W + 2],
                                             v23[:, 32:65, 0:W + 2])
                        return mb3

                # pre-allocate all 8 PSUM banks for this rep
                pss = {(b, ocb): [pspool.tile([128, 512], F32, tag="ps",
                                              name=f"ps{rep}_{b}_{ocb}_{g}")
                                  for g in range(2)]
                       for b in range(BPC) for ocb in range(NOB)}

                xb3s = {}
                xb3s[0] = emit_load(0)
                if NB > 1:
                    xb3s[1] = emit_load(1)

                def emit_warmup(xb3, n_mm):
                    # PE warmup: garbage matmuls into a bank the real taps
                    # later clear (start=True). Gated on this block's xb so
                    # they spread through the FIR lead-in, keeping HAM warm.
                    wout = pss[(0, 0)][0][:].rearrange("p (h w) -> p h w", w=OW)
                    rhs = xb3[:, 0:16, 0:32]
                    for _ in range(n_mm):
                        nc.tensor.matmul(wout, w_sb[:, 0:128], rhs,
                                         start=True, stop=True)

                mimg = {}
                for i in range(NB):
                    if rep == 0:
                        emit_warmup(xb3s[i], 12)
                    st = {"xb3": xb3s[i]}
                    ma3 = emit_fir_half(i, 0, st)
                    mb3 = emit_fir_half(i, 1, st)
                    mimg[blocks[i]] = (ma3, mb3)
                    if i + 2 < NB:
                        xb3s[i + 2] = emit_load(i + 2)

                # ---------- conv stage ----------
                # ic-chunk outermost (PE starts after the first chunk's FIR),
                # grp0 before grp1 (half A of the FIR lands first)
                for b in range(BPC):
                    for ci in range(NCH):
                        for grp in range(2):
                            for ocb in range(NOB):
                                for kh in range(KH):
                                    for q in range(KQ):
                                        widx = ((kh * KQ + q) * NCH + ci) \
                                            * NOB + ocb
                                        lhsT = w_sb[:, widx * 128:
                                                    (widx + 1) * 128]
                                        first = ci == 0 and kh == 0 and q == 0
                                        last = (ci == NCH - 1 and kh == KH - 1
                                                and q == KQ - 1)
                                        rhs = mimg[(b, ci)][grp][
                                            :, kh:kh + 31:2, q:q + 63:2]
                                        out3 = pss[(b, ocb)][grp][:].rearrange(
                                            "p (h w) -> p h w", w=OW)
                                        nc.tensor.matmul(out3, lhsT, rhs,
                                                         start=first, stop=last)
                    for ocb in range(NOB):
                        osb = opool.tile([128, OH * OW], F32, tag="osb",
                                         name=f"osb{rep}_{b}_{ocb}")
                        for grp in range(2):
                            nc.scalar.activation(
                                osb[:, grp * 512:(grp + 1) * 512],
                                pss[(b, ocb)][grp][:],
                                IDENT, bias=bias_sb[:, ocb:ocb + 1])
                        nc.sync.dma_start(
                            out=yout[b, ocb * 128:(ocb + 1) * 128, :, :]
                            .rearrange("c h w -> c (h w)"),
                            in_=osb[:])
    _split_excess_waits(nc)
    return nc


def prep_weights(w: np.ndarray) -> np.ndarray:
    """w [256,256,3,3] f32 -> [128, 48*128] bf16 lhsT tiles.
    Folds horizontal [1,1] and the 1/64 FIR normalization:
    w4[q] coefficients multiply M[2ow+q-1]."""
    w = np.asarray(w, np.float32)
    w4 = np.zeros((OC, C, KH, KQ), np.float32)
    w4[:, :, :, 0] = w[:, :, :, 0]
    w4[:, :, :, 1] = w[:, :, :, 0] + w[:, :, :, 1]
    w4[:, :, :, 2] = w[:, :, :, 1] + w[:, :, :, 2]
    w4[:, :, :, 3] = w[:, :, :, 2]
    w4 *= 1.0 / 64.0
    # -> [kh, q, c_chunk, ocb, ic(128), oc(128)]
    t = w4.reshape(NOB, 128, NCH, 128, KH, KQ).transpose(4, 5, 2, 0, 3, 1)
    t = np.ascontiguousarray(t).reshape(KH * KQ * NCH * NOB, 128, 128)
    return t.transpose(1, 0, 2).reshape(128, -1).astype(ml_dtypes.bfloat16)


_NC_CACHE: dict[int, bass.Bass] = {}


def _get_nc(reps: int = 1) -> bass.Bass:
    if reps not in _NC_CACHE:
        _NC_CACHE[reps] = build_program(reps)
    return _NC_CACHE[reps]


def make_in_maps(x: np.ndarray, w: np.ndarray, b: np.ndarray):
    wp = prep_weights(w)
    bp = np.asarray(b, np.float32).reshape(NOB, 128)
    return [
        {"x": np.ascontiguousarray(np.asarray(x, np.float32)[i * BPC:(i + 1) * BPC]),
         "w": wp, "b": bp}
        for i in range(N_CORES)
    ]


def kernel(x: np.ndarray, w: np.ndarray, b: np.ndarray) -> np.ndarray:
    nc = _get_nc(1)
    res = run_bass_kernel_spmd(nc, make_in_maps(x, w, b), list(range(N_CORES)))
    return np.concatenate([res.results[i]["y"] for i in range(N_CORES)],
                          axis=0).astype(np.float32)


# revision 17
# speedup vs baseline: 144.9315x; 51.7114x over previous
"""DownSample (depthwise FIR [1,3,3,1]^2/64 pad-2, then 3x3 stride-2 conv + bias)
as a Trainium2 Bass kernel, data-parallel over batch across 8 NeuronCores.

Per core (2 batch images), per (batch, ic-chunk) block:
  ACT : f32 -> bf16 ingest cast into a row-padded buffer
  DVE : vertical FIR [1,3,3,1] as three [1,1]-box cascades (all 2x mode)
        horizontal [1,2,1] as u = V[g-1]+V[g+1] (2x) and M = u + 2V
  ACT : the 2V scaled copy + PSUM evacuation with fused bias
  PE  : y[oc, oh, ow] = sum over (ic-chunk, kh in 3, q in 4) of
        w4[kh,q,ic,oc]^T @ M[ic, 2oh+kh, 2ow+q-1]   (stride-2 strided rhs APs)
        where w4 = conv_w(w, [1,1]) (horizontal [1,1] folded into weights),
        24 accumulating N=512 matmuls per PSUM bank, 8 banks.
Host folds [1,1]_h and the 1/64 FIR norm into w4.
"""

import numpy as np
import ml_dtypes

import concourse.bass as bass
import concourse.mybir as mybir
import concourse.tile as tile
from concourse.bass_utils import run_bass_kernel_spmd
from concourse.vector_clock import ScopedClock, VectorClock

# problem geometry (hardcoded per contract)
B_FULL, C, H, W = 16, 256, 64, 64
OC, OH, OW = 256, 32, 32
N_CORES = 8
BPC = B_FULL // N_CORES      # batches per core
KH, KQ = 3, 4                # folded conv taps (rows x cols)
NCH = C // 128               # input-channel chunks
NOB = OC // 128              # output-channel blocks
WV = W + 4                   # padded row width of V/u/v2/M buffers

F32 = mybir.dt.float32
BF16 = mybir.dt.bfloat16
COPY = mybir.ActivationFunctionType.Copy
IDENT = mybir.ActivationFunctionType.Identity


class SplitDrainTileContext(tile.TileContext):
    """walrus codegen caps sync-wait commands per instruction; the kernel-tail
    drain waits once per proc lane, which overflows once several DMA queues
    are used. Emit one single-wait SP nop per pending proc first; SP executes
    in order, so the drain itself then needs no waits."""

    def _drain_and_barrier(self, tick_clock, wait_clock):
        g = tick_clock.global_clock
        n = len(g)
        for p in range(n):
            if g[p] > 0:
                vec = [g[q] if q == p else 0 for q in range(n)]
                nop = self.nc.sync.nop()
                wait_clock.add_sem_waits(nop.ins, ScopedClock({None: VectorClock(vec)}))
        self.nc.sync.drain()
        self.nc.all_engine_barrier()
        assert self.sems is not None
        popped = self.nc._tile_sem_poison_stack.pop()
        assert popped is self._sem_poison
        self.nc.clear_and_free_semaphores(list(self.sems.allocated().values()))
        self.nc.all_engine_barrier()


def _split_excess_waits(nc: bass.Bass, max_waits: int = 1) -> None:
    """The TRN2 ISA (and walrus codegen) allows at most 2 sync-wait commands
    per instruction (1 for matmul), but Tile's wait assignment can attach more. Spill excess
    waits onto same-engine nops inserted immediately before the instruction —
    the engine executes them first, so the precondition set is identical."""
    for fn in nc.m.functions:
        for bb in fn.blocks:
            out = []
            changed = False
            for ins in bb.instructions:
                si = ins.sync_info
                waits = list(si.on_wait) if si else []
                if len(waits) > max_waits:
                    changed = True
                    excess, keep = waits[:-max_waits], waits[-max_waits:]
                    for k in range(0, len(excess), max_waits):
                        nop = mybir.InstNoOp(
                            name=f"{ins.name}-wsplit{k}", ins=[], outs=[],
                            engine=ins.engine)
                        nop.sync_info = mybir.SyncInfo(
                            on_wait=excess[k:k + max_waits], on_update=[])
                        nc.register_instruction(nop, overwrite=True)
                        out.append(nop)
                    ins.sync_info = mybir.SyncInfo(
                        on_wait=keep, on_update=list(si.on_update))
                out.append(ins)
            if changed:
                bb.instructions = out


def build_program(reps: int = 1, loop_n: int | None = None) -> bass.Bass:
    """One-core program; run SPMD on 8 cores. `reps` unrolls the body for
    steady-state timing (outputs are rewritten identically each rep);
    `loop_n` additionally wraps the body in a device-side For_i loop so the
    device time dominates host dispatch quanta when timing."""
    nc = bass.Bass()
    xin = nc.declare_dram_parameter("x", [BPC, C, H, W], F32, isOutput=False)
    win = nc.declare_dram_parameter("w", [128, KH * KQ * NCH * NOB * 128], BF16,
                                    isOutput=False)
    bin_ = nc.declare_dram_parameter("b", [NOB, 128], F32, isOutput=False)
    yout = nc.declare_dram_parameter("y", [BPC, OC, OH, OW], F32, isOutput=True)

    with SplitDrainTileContext(nc) as tc:
        with tc.tile_pool(name="const", bufs=1) as cpool, \
             tc.tile_pool(name="xf", bufs=4) as xfpool, \
             tc.tile_pool(name="fir", bufs=1) as f1pool, \
             tc.tile_pool(name="fir2", bufs=2) as f2pool, \
             tc.tile_pool(name="mimg", bufs=2 * NCH) as mpool, \
             tc.tile_pool(name="osb", bufs=2) as opool, \
             tc.tile_pool(name="ps", bufs=8, space="PSUM") as pspool:

            w_sb = cpool.tile([128, KH * KQ * NCH * NOB * 128], BF16)
            nc.sync.dma_start(out=w_sb[:], in_=win[:])
            bias_sb = cpool.tile([128, NOB], F32)
            nc.sync.dma_start(out=bias_sb[:], in_=bin_[:].rearrange("o p -> p o"))

            blocks = [(b, c) for b in range(BPC) for c in range(NCH)]
            NB = len(blocks)

            for rep in range(reps):
                _loop = None
                if loop_n is not None:
                    _loop = tc.For_i(0, loop_n, 1, hint_engines=(
                        mybir.EngineType.PE, mybir.EngineType.DVE,
                        mybir.EngineType.Activation))
                    _loop.__enter__()
                # x ingest: DMA + f32->bf16 cast, split in row-halves so the
                # first half's M image is ready fast (chunks spread over lanes)
                def emit_load(i):
                    b, c = blocks[i]
                    xb = f2pool.tile([128, (H + 4) * W], BF16, tag="xb",
                                     name=f"xb{rep}_{i}")
                    xb3 = xb[:].rearrange("p (h w) -> p h w", w=W)
                    nc.vector.memset(xb3[:, 0:2, :], 0.0)
                    nc.vector.memset(xb3[:, H + 2:H + 4, :], 0.0)
                    # chunk a: x rows 0..33 -> xb rows 2..35
                    # chunk b: x rows 34..63 -> xb rows 36..65
                    for (r0, nr) in ((0, 34), (34, 30)):
                        xf = xfpool.tile([128, nr * W], F32, tag="xf",
                                         name=f"xf{rep}_{i}_{r0}")
                        nc.sync.dma_start(
                            out=xf[:],
                            in_=xin[b, c * 128:(c + 1) * 128, r0:r0 + nr, :]
                            .rearrange("c h w -> c (h w)"),
                        )
                        nc.scalar.activation(
                            xb3[:, 2 + r0:2 + r0 + nr, :],
                            xf[:].rearrange("p (h w) -> p h w", w=W), COPY)
                    return xb3

                # FIR for one block, row-half `half` -> M tile for psum grp
                # `half`. Half A covers M rows 0..32, half B rows 32..64
                # (row 32 shared: grp0 reads Ma, grp1 reads Mb).
                # Buffers c1/c2/v/u/v2 are shared across halves (disjoint row
                # ranges); B reads a few boundary rows written by A.
                def emit_fir_half(i, half, st):
                    if half == 0:
                        c1 = f1pool.tile([128, (H + 3) * W], BF16, tag="c1",
                                         name=f"c1_{rep}_{i}")
                        c13 = c1[:].rearrange("p (h w) -> p h w", w=W)
                        # c1 rows 0..34
                        nc.vector.tensor_add(c13[:, 0:35, :], st["xb3"][:, 0:35, :],
                                             st["xb3"][:, 1:36, :])
                        c2 = f1pool.tile([128, (H + 2) * W], BF16, tag="c2",
                                         name=f"c2_{rep}_{i}")
                        c23 = c2[:].rearrange("p (h w) -> p h w", w=W)
                        # c2 rows 0..33
                        nc.vector.tensor_add(c23[:, 0:34, :], c13[:, 0:34, :],
                                             c13[:, 1:35, :])
                        vb = f2pool.tile([128, (H + 1) * WV], BF16, tag="v",
                                         name=f"v_{rep}_{i}")
                        v3 = vb[:].rearrange("p (h w) -> p h w", w=WV)
                        nc.vector.memset(v3[:, :, 0:2], 0.0)
                        nc.vector.memset(v3[:, :, W + 2:W + 4], 0.0)
                        # V rows 0..32 (cols 2..65 = data)
                        nc.vector.tensor_add(v3[:, 0:33, 2:2 + W],
                                             c23[:, 0:33, :], c23[:, 1:34, :])
                        v2b = f2pool.tile([128, (H + 1) * WV], BF16, tag="v2",
                                          name=f"v2_{rep}_{i}")
                        v23 = v2b[:].rearrange("p (h w) -> p h w", w=WV)
                        nc.scalar.activation(v23[:, 0:33, 0:W + 2],
                                             v3[:, 0:33, 1:W + 3], COPY, scale=2.0)
                        ub = f1pool.tile([128, (H + 1) * WV], BF16, tag="u",
                                         name=f"u_{rep}_{i}")
                        u3 = ub[:].rearrange("p (h w) -> p h w", w=WV)
                        nc.vector.tensor_add(u3[:, 0:33, 0:W + 2],
                                             v3[:, 0:33, 0:W + 2],
                                             v3[:, 0:33, 2:W + 4])
                        ma = mpool.tile([128, 33 * WV], BF16, tag="m",
                                        name=f"ma_{rep}_{i}")
                        ma3 = ma[:].rearrange("p (h w) -> p h w", w=WV)
                        nc.vector.tensor_add(ma3[:, :, 0:W + 2],
                                             u3[:, 0:33, 0:W + 2],
                                             v23[:, 0:33, 0:W + 2])
                        st.update(c13=c13, c23=c23, v3=v3, v23=v23, u3=u3)
                        return ma3
                    else:
                        c13, c23, v3, v23, u3 = (st["c13"], st["c23"], st["v3"],
                                                 st["v23"], st["u3"])
                        # c1 rows 35..66
                        nc.vector.tensor_add(c13[:, 35:67, :],
                                             st["xb3"][:, 35:67, :],
                                             st["xb3"][:, 36:68, :])
                        # c2 rows 34..65 (c1 row 34 from half A)
                        nc.vector.tensor_add(c23[:, 34:66, :], c13[:, 34:66, :],
                                             c13[:, 35:67, :])
                        # V rows 33..64 (c2 row 33 from half A)
                        nc.vector.tensor_add(v3[:, 33:65, 2:2 + W],
                                             c23[:, 33:65, :], c23[:, 34:66, :])
                        nc.scalar.activation(v23[:, 33:65, 0:W + 2],
                                             v3[:, 33:65, 1:W + 3], COPY,
                                             scale=2.0)
                        nc.vector.tensor_add(u3[:, 33:65, 0:W + 2],
                                             v3[:, 33:65, 0:W + 2],
                                             v3[:, 33:65, 2:W + 4])
                        mb = mpool.tile([128, 33 * WV], BF16, tag="m",
                                        name=f"mb_{rep}_{i}")
                        mb3 = mb[:].rearrange("p (h w) -> p h w", w=WV)
                        # M rows 32..64 (u/v2 row 32 from half A)
                        nc.vector.tensor_add(mb3[:, :, 0:W + 2],
                                             u3[:, 32:65, 0:W + 2],
                                             v23[:, 32:65, 0:W + 2])
                        return mb3

                # pre-allocate all 8 PSUM banks for this rep
                pss = {(b, ocb): [pspool.tile([128, 512], F32, tag="ps",
                                              name=f"ps{rep}_{b}_{ocb}_{g}")
                                  for g in range(2)]
                       for b in range(BPC) for ocb in range(NOB)}

                xb3s = {}
                xb3s[0] = emit_load(0)
                if NB > 1:
                    xb3s[1] = emit_load(1)

                def emit_warmup(xb3, n_mm):
                    # PE warmup: garbage matmuls into a bank the real taps
                    # later clear (start=True). Gated on this block's xb so
                    # they spread through the FIR lead-in, keeping HAM warm.
                    wout = pss[(0, 0)][0][:].rearrange("p (h w) -> p h w", w=OW)
                    rhs = xb3[:, 0:16, 0:32]
                    for _ in range(n_mm):
                        nc.tensor.matmul(wout, w_sb[:, 0:128], rhs,
                                         start=True, stop=True)

                mimg = {}
                for i in range(NB):
                    if rep == 0 and loop_n is None:
                        emit_warmup(xb3s[i], 12)
                    st = {"xb3": xb3s[i]}
                    ma3 = emit_fir_half(i, 0, st)
                    mb3 = emit_fir_half(i, 1, st)
                    mimg[blocks[i]] = (ma3, mb3)
                    if i + 2 < NB:
                        xb3s[i + 2] = emit_load(i + 2)

                # ---------- conv stage ----------
                # ic-chunk outermost (PE starts after the first chunk's FIR),
                # grp0 before grp1 (half A of the FIR lands first)
                for b in range(BPC):
                    for ci in range(NCH):
                        for grp in range(2):
                            for ocb in range(NOB):
                                for kh in range(KH):
                                    for q in range(KQ):
                                        widx = ((kh * KQ + q) * NCH + ci) \
                                            * NOB + ocb
                                        lhsT = w_sb[:, widx * 128:
                                                    (widx + 1) * 128]
                                        first = ci == 0 and kh == 0 and q == 0
                                        last = (ci == NCH - 1 and kh == KH - 1
                                                and q == KQ - 1)
                                        rhs = mimg[(b, ci)][grp][
                                            :, kh:kh + 31:2, q:q + 63:2]
                                        out3 = pss[(b, ocb)][grp][:].rearrange(
                                            "p (h w) -> p h w", w=OW)
                                        nc.tensor.matmul(out3, lhsT, rhs,
                                                         start=first, stop=last)
                    for ocb in range(NOB):
                        osb = opool.tile([128, OH * OW], F32, tag="osb",
                                         name=f"osb{rep}_{b}_{ocb}")
                        for grp in range(2):
                            nc.scalar.activation(
                                osb[:, grp * 512:(grp + 1) * 512],
                                pss[(b, ocb)][grp][:],
                                IDENT, bias=bias_sb[:, ocb:ocb + 1])
                        nc.sync.dma_start(
                            out=yout[b, ocb * 128:(ocb + 1) * 128, :, :]
                            .rearrange("c h w -> c (h w)"),
                            in_=osb[:])
                if _loop is not None:
                    _loop.__exit__(None, None, None)
    _split_excess_waits(nc)
    return nc


def prep_weights(w: np.ndarray) -> np.ndarray:
    """w [256,256,3,3] f32 -> [128, 48*128] bf16 lhsT tiles.
    Folds horizontal [1,1] and the 1/64 FIR normalization:
    w4[q] coefficients multiply M[2ow+q-1]."""
    w = np.asarray(w, np.float32)
    w4 = np.zeros((OC, C, KH, KQ), np.float32)
    w4[:, :, :, 0] = w[:, :, :, 0]
    w4[:, :, :, 1] = w[:, :, :, 0] + w[:, :, :, 1]
    w4[:, :, :, 2] = w[:, :, :, 1] + w[:, :, :, 2]
    w4[:, :, :, 3] = w[:, :, :, 2]
    w4 *= 1.0 / 64.0
    # -> [kh, q, c_chunk, ocb, ic(128), oc(128)]
    t = w4.reshape(NOB, 128, NCH, 128, KH, KQ).transpose(4, 5, 2, 0, 3, 1)
    t = np.ascontiguousarray(t).reshape(KH * KQ * NCH * NOB, 128, 128)
    return t.transpose(1, 0, 2).reshape(128, -1).astype(ml_dtypes.bfloat16)


_NC_CACHE: dict = {}


def _get_nc(reps: int = 1, loop_n: int | None = None) -> bass.Bass:
    key = (reps, loop_n)
    if key not in _NC_CACHE:
        _NC_CACHE[key] = build_program(reps, loop_n)
    return _NC_CACHE[key]


def make_in_maps(x: np.ndarray, w: np.ndarray, b: np.ndarray):
    wp = prep_weights(w)
    bp = np.asarray(b, np.float32).reshape(NOB, 128)
    return [
        {"x": np.ascontiguousarray(np.asarray(x, np.float32)[i * BPC:(i + 1) * BPC]),
         "w": wp, "b": bp}
        for i in range(N_CORES)
    ]


def kernel(x: np.ndarray, w: np.ndarray, b: np.ndarray) -> np.ndarray:
    nc = _get_nc(1)
    res = run_bass_kernel_spmd(nc, make_in_maps(x, w, b), list(range(N_CORES)))
    return np.concatenate([res.results[i]["y"] for i in range(N_CORES)],
                          axis=0).astype(np.float32)


def make_runner(nc, in_maps):
    """Hoisted version of bass2jax.run_bass_via_pjrt: build the sharded jit
    once with device-resident operands; returns (run_async, block) for
    throughput timing and a `fetch` to materialize outputs."""
    import jax
    from concourse import bass2jax
    from jax.sharding import Mesh, PartitionSpec, NamedSharding
    from jax.experimental.shard_map import shard_map

    bass2jax.install_neuronx_cc_hook()
    partition_name = nc.partition_id_tensor.name if nc.partition_id_tensor else None
    in_names, out_names, out_avals, zero_outs = [], [], [], []
    for alloc in nc.m.functions[0].allocations:
        if not isinstance(alloc, mybir.MemoryLocationSet):
            continue
        name = alloc.memorylocations[0].name
        if alloc.kind == "ExternalInput":
            if name != partition_name:
                in_names.append(name)
        elif alloc.kind == "ExternalOutput":
            shape = tuple(alloc.tensor_shape)
            dtype = mybir.dt.np(alloc.dtype)
            out_names.append(name)
            out_avals.append(jax.core.ShapedArray(shape, dtype))
            zero_outs.append(np.zeros(shape, dtype))
    n_params = len(in_names)
    all_in_names = list(in_names) + list(out_names)
    if partition_name is not None:
        all_in_names.append(partition_name)

    def _body(*args):
        operands = list(args)
        if partition_name is not None:
            operands.append(bass2jax.partition_id_tensor())
        return tuple(bass2jax._bass_exec_p.bind(
            *operands,
            out_avals=tuple(out_avals),
            in_names=tuple(all_in_names),
            out_names=tuple(out_names),
            lowering_input_output_aliases=(),
            sim_require_finite=True,
            sim_require_nnan=True,
            nc=nc,
        ))

    devices = jax.devices()[:N_CORES]
    mesh = Mesh(np.asarray(devices), ("core",))
    sharded = jax.jit(
        shard_map(_body, mesh=mesh,
                  in_specs=(PartitionSpec("core"),) * (n_params + len(out_names)),
                  out_specs=(PartitionSpec("core"),) * len(out_names),
                  check_rep=False),
        donate_argnums=(), keep_unused=True)
    sh = NamedSharding(mesh, PartitionSpec("core"))
    dev_in = [jax.device_put(np.concatenate(
        [np.asarray(in_maps[c][nm]) for c in range(N_CORES)], axis=0), sh)
        for nm in in_names]
    dev_zeros = [jax.device_put(
        np.zeros((N_CORES * z.shape[0], *z.shape[1:]), z.dtype), sh)
        for z in zero_outs]

    def run_async():
        return sharded(*dev_in, *dev_zeros)

    def block(out):
        import jax
        jax.block_until_ready(out)

    return run_async, block, out_names
